# revision 8
# baseline (speedup 1.0000x reference)
"""GCN (3-layer, symmetric-norm) + global mean pool + linear classifier on 8 trn2 cores.

Strategy:
  - Partition nodes by graph id into 8 contiguous ranges (16 graphs/core) so pooling
    is local; each core owns edges whose dst falls in its range, grouped by 256-node
    dst superblocks.
  - Per superblock: one indirect-DMA gather of all referenced table rows (512 B/row),
    then per 128-edge tile a one-hot matrix M[e,d] = norm_e * (dst_e == d) built by a
    single dual-op DVE tensor_scalar, and a PE matmul accumulating
    aggT[f, d] += G[e,f]^T M[e,d] in PSUM.  All matmuls are float32r (1 cycle/row at
    free-dim >= 256; walrus requires both operands f32r).
  - z^T[hid, d] = W^T aggT (weights stationary), h^T = relu(z^T + b) as one ACT op
    (bias is per-partition in this orientation), then a PE transpose back to
    node-major [d, hid] for the table write / pooling.
  - AllGather of the [R,128] table shard after layers 1 and 2 (the only collective).
  - Layer 3 skips relu + table write; pooling via a 0/1 graph-membership matmul
    accumulated over blocks, then the classifier matmul.  Host concatenates the
    8 [16,10] outputs.
"""

import numpy as np

import concourse.bass as bass
import concourse.bacc as bacc
import concourse.tile as tile
from concourse import mybir
from concourse import bass_utils

P = 128
SBW = 256          # dst superblock width (agg matmul free dim)
N_CORES = 8
GROUP_SB = 2       # superblocks per indirect gather

# float32r: 1 cycle/row matmul when the moving free dim >= 256 (vs 4 for float32).
# Sim treats it as exact fp32; HW numerics are slightly reduced.  Set to
# mybir.dt.float32 for the exact-but-4x-slower fallback.
MM_F32 = mybir.dt.float32r


def _r(ap):
    if MM_F32 == mybir.dt.float32:
        return ap
    return ap.bitcast(MM_F32)


def preprocess(x, edge_index, batch_ids, w1, b1, w2, b2, w3, b3, lin_w, lin_b):
    """All index math on CPU.  Returns (meta dict, per-core input maps)."""
    x = np.asarray(x, np.float32)
    ei = np.asarray(edge_index).astype(np.int64)
    bid = np.asarray(batch_ids).astype(np.int64)
    n, f = x.shape
    lin_w = np.asarray(lin_w, np.float32)
    n_cls = lin_w.shape[1]
    n_graphs = 128
    gpc = n_graphs // N_CORES

    # --- gcn_norm with self loops ---
    src = np.concatenate([ei[0], np.arange(n, dtype=np.int64)])
    dst = np.concatenate([ei[1], np.arange(n, dtype=np.int64)])
    deg = np.bincount(dst, minlength=n).astype(np.float64)
    dinv = np.where(deg > 0, 1.0 / np.sqrt(np.maximum(deg, 1.0)), 0.0).astype(np.float32)
    enorm_all = (dinv[src] * dinv[dst]).astype(np.float32)

    # --- node partition by graph id ---
    bounds = np.searchsorted(bid, np.arange(0, n_graphs + 1, gpc))
    nk = np.diff(bounds)
    SB = int(np.ceil(nk.max() / SBW))   # superblocks per core
    B = SB * 2                          # 128-blocks per core
    R = SB * SBW                        # rows per core region
    NT = N_CORES * R
    core_of_node = np.searchsorted(bounds, np.arange(n), side="right") - 1
    ppos = (core_of_node * R + (np.arange(n) - bounds[core_of_node])).astype(np.int64)

    # --- layer-1 table (raw x, padded layout) ---
    t1 = np.zeros((NT, f), np.float32)
    t1[ppos] = x

    # --- edge partition by (dst core, dst superblock) ---
    ecore = core_of_node[dst]
    edloc = dst - bounds[ecore]
    esb = edloc // SBW
    key = ecore * SB + esb
    order = np.argsort(key, kind="stable")
    s_src_p = ppos[src[order]].astype(np.int32)
    s_dloc = (edloc[order] % SBW).astype(np.float32)
    s_norm = enorm_all[order]

    cnt = np.bincount(key, minlength=N_CORES * SB).reshape(N_CORES, SB)
    Tsb = np.maximum(np.ceil(cnt.max(axis=0) / P).astype(np.int64), 1)
    TO = np.concatenate([[0], np.cumsum(Tsb)]).astype(np.int64)
    TT = int(TO[-1])

    gidx = np.zeros((N_CORES, P, TT), np.int32)
    dloc = np.full((N_CORES, P, TT), -1.0, np.float32)
    enorm = np.zeros((N_CORES, P, TT), np.float32)
    starts = np.concatenate([[0], np.cumsum(cnt.ravel())])
    for k in range(N_CORES):
        for b in range(SB):
            i0, i1 = starts[k * SB + b], starts[k * SB + b + 1]
            c = i1 - i0
            if c == 0:
                continue
            j = np.arange(c)
            col = TO[b] + j // P
            row = j % P
            gidx[k, row, col] = s_src_p[i0:i1]
            dloc[k, row, col] = s_dloc[i0:i1]
            enorm[k, row, col] = s_norm[i0:i1]

    # --- graph membership masks (per 128-block) and pool scaling ---
    gmask = np.zeros((N_CORES, P, B * gpc), np.float32)
    for k in range(N_CORES):
        idx = np.arange(nk[k])
        nodes = bounds[k] + idx
        g_local = bid[nodes] - k * gpc
        gmask[k, idx % P, (idx // P) * gpc + g_local] = 1.0
    cnts = np.bincount(bid, minlength=n_graphs).astype(np.float32)
    recip = 1.0 / np.maximum(cnts, 1.0)

    # --- constant packing: meta_f32 [P, C] ---
    w1 = np.asarray(w1, np.float32); w2 = np.asarray(w2, np.float32)
    w3 = np.asarray(w3, np.float32)
    b1 = np.asarray(b1, np.float32); b2 = np.asarray(b2, np.float32)
    b3 = np.asarray(b3, np.float32)
    lin_b = np.asarray(lin_b, np.float32)
    hid = w1.shape[1]

    off = {}
    cols = []

    def add(name, arr):
        if arr.ndim == 2:
            arr = np.broadcast_to(arr[None], (N_CORES,) + arr.shape)
        off[name] = sum(a.shape[2] for a in cols)
        cols.append(np.ascontiguousarray(arr).astype(np.float32))

    add("dloc", dloc)
    add("enorm", enorm)
    add("bias", np.stack([b1, b2, b3], axis=1))  # [hid, 3] column per layer
    add("lb", np.broadcast_to(lin_b[None, :], (P, n_cls)))
    rc = np.zeros((N_CORES, P, 1), np.float32)
    for k in range(N_CORES):
        rc[k, :gpc, 0] = recip[k * gpc:(k + 1) * gpc]
    add("rc", rc)
    add("iota", np.broadcast_to(np.arange(SBW, dtype=np.float32)[None, :], (P, SBW)))
    add("ident", np.eye(P, dtype=np.float32))
    meta_f32 = np.concatenate(cols, axis=2)

    # constants consumed by fp32r matmuls live in their own tensor so the
    # whole tensor can be declared float32r (walrus: fp32r matmul operands
    # must be produced as fp32r).
    woff = {}
    wcols = []

    def addw(name, arr):
        if arr.ndim == 2:
            arr = np.broadcast_to(arr[None], (N_CORES,) + arr.shape)
        woff[name] = sum(a.shape[2] for a in wcols)
        wcols.append(np.ascontiguousarray(arr).astype(np.float32))

    addw("w1", w1); addw("w2", w2); addw("w3", w3)
    addw("gmask", gmask)
    addw("lw", lin_w)
    meta_w = np.concatenate(wcols, axis=2)

    meta = dict(
        n=n, f=f, hid=hid, n_cls=n_cls, gpc=gpc, SB=SB, B=B, R=R, NT=NT,
        Tsb=Tsb, TO=TO, TT=TT, off=off, C=meta_f32.shape[2],
        woff=woff, CW=meta_w.shape[2],
    )
    in_maps = []
    for k in range(N_CORES):
        in_maps.append({
            "t1": t1,
            "gidx": np.ascontiguousarray(gidx[k]),
            "mf": np.ascontiguousarray(meta_f32[k]),
            "mw": np.ascontiguousarray(meta_w[k]),
        })
    return meta, in_maps


def build_program(meta):
    f32 = mybir.dt.float32
    f32r = MM_F32
    i32 = mybir.dt.int32
    SB, B, R, NT, TT = meta["SB"], meta["B"], meta["R"], meta["NT"], meta["TT"]
    Tsb, TO = meta["Tsb"], meta["TO"]
    C, off = meta["C"], meta["off"]
    CW, woff = meta["CW"], meta["woff"]
    hid, n_cls, gpc, f = meta["hid"], meta["n_cls"], meta["gpc"], meta["f"]
    Relu = mybir.ActivationFunctionType.Relu
    Ident = mybir.ActivationFunctionType.Identity

    nc = bacc.Bacc("TRN2", target_bir_lowering=False, debug=False,
                   num_devices=N_CORES)
    t1_d = nc.dram_tensor("t1", [NT, f], f32r, kind="ExternalInput")
    gidx_d = nc.dram_tensor("gidx", [P, TT], i32, kind="ExternalInput")
    mf_d = nc.dram_tensor("mf", [P, C], f32, kind="ExternalInput")
    mw_d = nc.dram_tensor("mw", [P, CW], f32r, kind="ExternalInput")
    out_d = nc.dram_tensor("out", [gpc, n_cls], f32, kind="ExternalOutput")

    groups = [list(range(g, min(g + GROUP_SB, SB)))
              for g in range(0, SB, GROUP_SB)]
    max_tg = max(int(sum(Tsb[b] for b in grp)) for grp in groups)

    with tile.TileContext(nc) as tc:
        with (
            tc.tile_pool(name="const", bufs=1) as cpool,
            tc.tile_pool(name="gath", bufs=2) as gpool,
            tc.tile_pool(name="m", bufs=6) as mpool,
            tc.tile_pool(name="work", bufs=4) as wpool,
            tc.tile_pool(name="ps", bufs=2, space="PSUM") as pspool,
            tc.tile_pool(name="pool_ps", bufs=1, space="PSUM") as ppool,
            tc.tile_pool(name="dram", bufs=1, space="DRAM") as dpool,
        ):
            mf = cpool.tile([P, C], f32, tag="mf")
            nc.sync.dma_start(mf[:], mf_d[:])
            mw = cpool.tile([P, CW], f32r, tag="mw")
            nc.sync.dma_start(mw[:], mw_d[:])
            gidx = cpool.tile([P, TT], i32, tag="gidx")
            nc.sync.dma_start(gidx[:], gidx_d[:])

            ag_in = [dpool.tile([R, f], f32r, tag=f"ag_in{l}", name=f"ag_in{l}")
                     for l in range(2)]
            ag_out = [dpool.tile([NT, f], f32r, tag=f"ag_out{l}", name=f"ag_out{l}",
                                 addr_space="Shared")
                      for l in range(2)]

            iota_ap = mf[:, off["iota"]:off["iota"] + SBW]
            ident_ap = mf[:, off["ident"]:off["ident"] + P]
            pooled = ppool.tile([P, gpc], f32, tag="pooled")

            tables = [t1_d[:, :], ag_out[0][:, :], ag_out[1][:, :]]
            w_offs = [woff["w1"], woff["w2"], woff["w3"]]

            for L in range(3):
                table = tables[L]
                w_ap = mw[:, w_offs[L]:w_offs[L] + hid]
                bias_ap = mf[:, off["bias"] + L:off["bias"] + L + 1]
                for grp in groups:
                    tg = int(sum(Tsb[b] for b in grp))
                    c0 = int(TO[grp[0]])
                    gt = gpool.tile([P, max_tg * P], f32r, tag="gath")
                    nc.gpsimd.indirect_dma_start(
                        out=gt[:, :tg * P],
                        out_offset=None,
                        in_=table,
                        in_offset=bass.IndirectOffsetOnAxis(
                            ap=gidx[:, c0:c0 + tg], axis=0),
                    )
                    for b in grp:
                        aggT = pspool.tile([P, SBW], f32, tag="aggT")  # [f, d]
                        for t in range(int(Tsb[b])):
                            col = int(TO[b]) + t
                            lc = col - c0
                            m = mpool.tile([P, SBW], f32r, tag="m")
                            nc.vector.tensor_scalar(
                                out=m[:], in0=iota_ap,
                                scalar1=mf[:, off["dloc"] + col:off["dloc"] + col + 1],
                                scalar2=mf[:, off["enorm"] + col:off["enorm"] + col + 1],
                                op0=mybir.AluOpType.is_equal,
                                op1=mybir.AluOpType.mult)
                            nc.tensor.matmul(
                                out=aggT[:],
                                lhsT=gt[:, lc * P:(lc + 1) * P],
                                rhs=m[:],
                                start=(t == 0), stop=(t == int(Tsb[b]) - 1))
                        aggs = wpool.tile([P, SBW], f32r, tag="aggs")
                        nc.vector.tensor_copy(out=aggs[:], in_=aggT[:])
                        zT = pspool.tile([P, SBW], f32, tag="zT")  # [hid, d]
                        nc.tensor.matmul(out=zT[:], lhsT=w_ap, rhs=aggs[:],
                                         start=True, stop=True)
                        hT = wpool.tile([P, SBW], f32, tag="hT")
                        nc.scalar.activation(
                            out=hT[:], in_=zT[:], func=(Relu if L < 2 else Ident),
                            bias=bias_ap)
                        for half in range(2):
                            blk = b * 2 + half  # 128-block index
                            hn = pspool.tile([P, hid], f32, tag="hn")  # [d, hid]
                            nc.tensor.transpose(
                                out=hn[:],
                                in_=hT[:, half * P:(half + 1) * P],
                                identity=ident_ap)
                            hs = wpool.tile([P, hid], f32r, tag="hs")
                            nc.vector.tensor_copy(out=hs[:], in_=hn[:])
                            if L < 2:
                                nc.sync.dma_start(
                                    ag_in[L][blk * P:(blk + 1) * P, :], hs[:])
                            else:
                                gm = woff["gmask"] + blk * gpc
                                nc.tensor.matmul(
                                    out=pooled[:], lhsT=hs[:],
                                    rhs=mw[:, gm:gm + gpc],
                                    start=(blk == 0), stop=(blk == B - 1))
                if L < 2:
                    nc.gpsimd.collective_compute(
                        "AllGather", mybir.AluOpType.bypass,
                        replica_groups=[list(range(N_CORES))],
                        ins=[ag_in[L].opt()], outs=[ag_out[L].opt()])

            # classifier
            ps = wpool.tile([P, gpc], f32r, tag="pooled_sb")
            nc.vector.tensor_copy(out=ps[:], in_=pooled[:])
            lg = ppool.tile([gpc, n_cls], f32, tag="lg")
            nc.tensor.matmul(out=lg[:], lhsT=ps[:],
                             rhs=mw[:, woff["lw"]:woff["lw"] + n_cls],
                             start=True, stop=True)
            l1 = wpool.tile([gpc, n_cls], f32, tag="l1")
            nc.vector.tensor_scalar(
                out=l1[:], in0=lg[:],
                scalar1=mf[:gpc, off["rc"]:off["rc"] + 1], scalar2=None,
                op0=mybir.AluOpType.mult)
            l2 = wpool.tile([gpc, n_cls], f32, tag="l2")
            nc.vector.tensor_tensor(
                out=l2[:], in0=l1[:], in1=mf[:gpc, off["lb"]:off["lb"] + n_cls],
                op=mybir.AluOpType.add)
            nc.sync.dma_start(out_d[:, :], l2[:])

    nc.compile()
    return nc


LAST_RESULT = None  # BassKernelResults of the most recent run (for profiling)


def kernel(**inputs):
    global LAST_RESULT
    meta, in_maps = preprocess(**inputs)
    nc = build_program(meta)
    res = bass_utils.run_bass_kernel_spmd(
        nc, in_maps, core_ids=list(range(N_CORES)))
    LAST_RESULT = res
    outs = [res.results[k]["out"] for k in range(N_CORES)]
    return np.concatenate(outs, axis=0).astype(np.float32)
